# revision 29
# baseline (speedup 1.0000x reference)
"""BF15IntLinear on 8 TRN2 NeuronCores.

Math: the reference quantizes x to "BF15" (truncate |x| toward zero to 6
explicit mantissa bits), W to truncated-bf16 (7 explicit bits), then does
an integer shift-align matmul whose result matches an exact
fp32-accumulated matmul of the quantized values to ~1e-5 relative — far
below the final bf16-cast ulp.  Both quantized operands are exactly
representable in bf16, and "truncate fp32 toward zero to bf16" is
literally "take the high uint16 of the fp32 word".

Kernel (per core; the 512x1024x1024 problem is sharded 2 M-groups x 4
N-groups):
  - fp32 operand shards are loaded with one DMA per row-tile, split
    across the two HWDGE trigger engines (sync / scalar) whose queue
    rings run concurrently (~200 GB/s each)
  - TensorE transposes read the hi-uint16 lane of the fp32 tiles via
    stride-2 bf16 access patterns — load-time truncate-to-bf16
    quantization for free; 36 dummy transposes of the identity run during
    the DMA phase to hold the HAM clock gate open (2.4 GHz) for the real
    matmul work
  - the PSUM->SBUF copy of the x tiles is a fused bitwise-AND 0xFFFE
    (clears the 7th mantissa bit -> BF15); W copies are plain; all on DVE,
    batched over kb-pairs
  - 16 bf16 matmuls (N=256 moving) accumulate into two PSUM fp32 banks
  - bias (host-replicated to 128 partitions) add + cast to bf16 (DVE),
    stores split across both trigger engines
"""

import numpy as np
import ml_dtypes

import concourse.bass as bass
import concourse.bacc as bacc
import concourse.mybir as mybir
from concourse import tile
from concourse.bass_utils import run_bass_kernel_spmd

# Problem shape (hardcoded per contract): x [4,128,1024] f32,
# weight [1024,1024] f32, bias [1024] f32 -> out [4,128,1024] bf16.
M, K, N = 512, 1024, 1024
M_GROUPS, N_GROUPS = 2, 4
M_SH, N_SH = M // M_GROUPS, N // N_GROUPS  # 256, 256
KB = K // 128  # 8 k-blocks
RT = M_SH // 128  # row-tiles per operand shard (2)
KH = K // 2  # DMA K-half
N_WARM = 36  # dummy PE transposes to hold the HAM clock gate open

_CACHE: dict = {}


def _build_nc():
    dt = mybir.dt
    nc = bacc.Bacc("TRN2", debug=False, target_bir_lowering=False)
    x_d = nc.dram_tensor("x", [M_SH, K], dt.float32, kind="ExternalInput")
    w_d = nc.dram_tensor("w", [N_SH, K], dt.float32, kind="ExternalInput")
    b_d = nc.dram_tensor("b", [128, N_SH], dt.float32, kind="ExternalInput")
    y_d = nc.dram_tensor("y", [M_SH, N_SH], dt.bfloat16, kind="ExternalOutput")
    warm_d = nc.dram_tensor("warm", [1, 128], dt.bfloat16, kind="ExternalOutput")

    with tile.TileContext(nc) as tc:
        with (
            tc.tile_pool(name="sb", bufs=1) as pool,
            tc.tile_pool(name="ps", bufs=2, space=bass.MemorySpace.PSUM) as psum,
            tc.tile_pool(name="acc", bufs=1, space=bass.MemorySpace.PSUM) as psacc,
        ):
            # identity built on-chip (gpsimd is otherwise idle, so this
            # completes ~3us before any DMA data): 0-fill, 1.0 diagonal
            idt = pool.tile([128, 128], dt.bfloat16, tag="idt")
            nc.gpsimd.memset(idt[:, :], 0.0)
            nc.gpsimd.affine_select(
                idt[:, :], idt[:, :], [[1, 128]],
                compare_op=mybir.AluOpType.not_equal, fill=1.0,
                base=0, channel_multiplier=-1,
            )

            # PE warmup: dummy transposes with no DMA deps — they run during
            # the load phase and hold the HAM clock gate open.  Kept alive
            # via a tiny DMA'd output.
            wps = psum.tile([128, 128], dt.bfloat16, tag="wps", bufs=1)
            for _ in range(N_WARM):
                nc.tensor.transpose(wps[:, :], idt[:, :], idt[:, :])
            wsb = pool.tile([1, 128], dt.bfloat16, tag="wsb")
            nc.vector.tensor_copy(wsb[0:1, :], wps[0:1, :])
            nc.scalar.dma_start(out=warm_d[:, :], in_=wsb[0:1, :])

            # loads: sync- and scalar-issued HWDGE DMAs use different queue
            # rings that run concurrently (~200 GB/s each) — split each
            # operand across both rings by row-tile, x before w
            xf = pool.tile([128, RT, K], dt.float32, tag="xf")
            wf = pool.tile([128, RT, K], dt.float32, tag="wf")
            x_src = x_d.ap().rearrange("(t p) k -> p t k", p=128)
            w_src = w_d.ap().rearrange("(t p) k -> p t k", p=128)
            nc.sync.dma_start(out=xf[:, 0:1, :], in_=x_src[:, 0:1, :])
            nc.scalar.dma_start(out=xf[:, 1:2, :], in_=x_src[:, 1:2, :])
            nc.sync.dma_start(out=wf[:, 0:1, :], in_=w_src[:, 0:1, :])
            nc.scalar.dma_start(out=wf[:, 1:2, :], in_=w_src[:, 1:2, :])
            bias_all = pool.tile([128, N_SH], dt.float32, tag="bias_all")
            nc.sync.dma_start(out=bias_all[:, :], in_=b_d[:, :])

            # hi-u16 lane views = truncated-bf16 bit patterns
            xhi = xf[:, :, :].bitcast(dt.bfloat16).rearrange(
                "p t (k two) -> p t k two", two=2
            )
            whi = wf[:, :, :].bitcast(dt.bfloat16).rearrange(
                "p t (k two) -> p t k two", two=2
            )

            # transpose hi-lanes to K-partition-major; phase-ordered so each
            # K-half's work starts as soon as its DMA lands
            xt = [None] * (KB // 2)
            wt = [None] * (KB // 2)
            acc = [
                psacc.tile([128, N_SH], dt.float32, tag=f"acc{mb}", name=f"acc{mb}")
                for mb in range(RT)
            ]

            # kb-pair batched transposes: 4 PE transposes per PSUM tile and
            # ONE DVE copy per pair (halves the DVE per-op overhead)
            def transpose_pair(kp, hi_view, dst_list, tag, masked):
                tk = pool.tile([128, 2, RT, 128], dt.bfloat16,
                               tag=f"{tag}{kp}", name=f"{tag}{kp}")
                pt = psum.tile([128, 2, RT, 128], dt.bfloat16, tag=f"pt_{tag}",
                               name=f"pt_{tag}{kp}", bufs=2)
                for i in range(2):
                    kb = kp * 2 + i
                    for t in range(RT):
                        nc.tensor.transpose(
                            pt[:, i, t, :],
                            hi_view[:, t, kb * 128:(kb + 1) * 128, 1],
                            idt[:, :],
                        )
                if masked:
                    # fused copy + BF15 mask (clear mantissa bit 7)
                    nc.vector.tensor_scalar(
                        out=tk[:, :, :, :].bitcast(dt.uint16),
                        in0=pt[:, :, :, :].bitcast(dt.uint16),
                        scalar1=0xFFFE, scalar2=None,
                        op0=mybir.AluOpType.bitwise_and,
                    )
                else:
                    nc.vector.tensor_copy(tk[:, :, :, :], pt[:, :, :, :])
                dst_list[kp] = tk

            for kp in range(KB // 2):
                transpose_pair(kp, xhi, xt, "xt", masked=True)
            for kp in range(KB // 2):
                transpose_pair(kp, whi, wt, "wt", masked=False)
            for kb in range(KB):
                kp, i = divmod(kb, 2)
                for mb in range(RT):
                    nc.tensor.matmul(
                        acc[mb][:, :],
                        xt[kp][:, i, mb, :],
                        wt[kp][:, i, :, :],
                        start=(kb == 0),
                        stop=(kb == KB - 1),
                    )

            # epilogue + store, one per M-block on separate trigger queues
            ysb = pool.tile([128, RT, N_SH], dt.bfloat16, tag="ysb")
            y_dst = y_d.ap().rearrange("(mb p) n -> p mb n", p=128)
            for mb in range(RT):
                nc.vector.tensor_tensor(
                    out=ysb[:, mb, :], in0=acc[mb][:, :], in1=bias_all[:, :],
                    op=mybir.AluOpType.add,
                )
                eng = nc.scalar if mb == 0 else nc.sync
                eng.dma_start(out=y_dst[:, mb, :], in_=ysb[:, mb, :])

    nc.compile()
    return nc


def get_nc():
    if "nc" not in _CACHE:
        _CACHE["nc"] = _build_nc()
    return _CACHE["nc"]


def make_in_maps(x: np.ndarray, weight: np.ndarray, bias: np.ndarray):
    x2d = np.ascontiguousarray(np.asarray(x).reshape(M, K), dtype=np.float32)
    w = np.ascontiguousarray(np.asarray(weight), dtype=np.float32)
    b = np.ascontiguousarray(np.asarray(bias), dtype=np.float32)
    in_maps = []
    for c in range(M_GROUPS * N_GROUPS):
        mi, ni = divmod(c, N_GROUPS)
        bs = np.ascontiguousarray(
            np.broadcast_to(b[ni * N_SH:(ni + 1) * N_SH], (128, N_SH))
        )
        in_maps.append({
            "x": np.ascontiguousarray(x2d[mi * M_SH:(mi + 1) * M_SH]),
            "w": np.ascontiguousarray(w[ni * N_SH:(ni + 1) * N_SH]),
            "b": bs,
        })
    return in_maps


def assemble(results) -> np.ndarray:
    y2d = np.empty((M, N), dtype=ml_dtypes.bfloat16)
    for c in range(M_GROUPS * N_GROUPS):
        mi, ni = divmod(c, N_GROUPS)
        y2d[mi * M_SH:(mi + 1) * M_SH, ni * N_SH:(ni + 1) * N_SH] = results[c]["y"]
    return y2d.reshape(4, 128, N)


def kernel(x: np.ndarray, weight: np.ndarray, bias: np.ndarray) -> np.ndarray:
    nc = get_nc()
    in_maps = make_in_maps(x, weight, bias)
    res = run_bass_kernel_spmd(nc, in_maps, core_ids=list(range(8)))
    return assemble(res.results)


# revision 30
# speedup vs baseline: 1.1664x; 1.1664x over previous
"""BF15IntLinear on 8 TRN2 NeuronCores.

Math: the reference quantizes x to "BF15" (truncate |x| toward zero to 6
explicit mantissa bits), W to truncated-bf16 (7 explicit bits), then does
an integer shift-align matmul whose result matches an exact
fp32-accumulated matmul of the quantized values to ~1e-5 relative — far
below the final bf16-cast ulp.  Both quantized operands are exactly
representable in bf16, and "truncate fp32 toward zero to bf16" is
literally "take the high uint16 of the fp32 word".

Kernel (per core; the 512x1024x1024 problem is sharded 2 M-groups x 4
N-groups):
  - fp32 operand shards are loaded with one DMA per row-tile, split
    across the two HWDGE trigger engines (sync / scalar) whose queue
    rings run concurrently (~200 GB/s each)
  - TensorE transposes read the hi-uint16 lane of the fp32 tiles via
    stride-2 bf16 access patterns — load-time truncate-to-bf16
    quantization for free; 36 dummy transposes of the identity run during
    the DMA phase to hold the HAM clock gate open (2.4 GHz) for the real
    matmul work
  - the PSUM->SBUF copy of the x tiles is a fused bitwise-AND 0xFFFE
    (clears the 7th mantissa bit -> BF15); W copies are plain; all on DVE,
    batched over kb-pairs
  - 16 bf16 matmuls (N=256 moving) accumulate into two PSUM fp32 banks
  - bias (host-replicated to 128 partitions) add + cast to bf16 (DVE),
    stores split across both trigger engines
"""

import numpy as np
import ml_dtypes

import concourse.bass as bass
import concourse.bacc as bacc
import concourse.mybir as mybir
from concourse import tile
from concourse.bass_utils import run_bass_kernel_spmd

# Problem shape (hardcoded per contract): x [4,128,1024] f32,
# weight [1024,1024] f32, bias [1024] f32 -> out [4,128,1024] bf16.
M, K, N = 512, 1024, 1024
M_GROUPS, N_GROUPS = 2, 4
M_SH, N_SH = M // M_GROUPS, N // N_GROUPS  # 256, 256
KB = K // 128  # 8 k-blocks
RT = M_SH // 128  # row-tiles per operand shard (2)
KH = K // 2  # DMA K-half
N_WARM = 36  # dummy PE transposes to hold the HAM clock gate open

_CACHE: dict = {}


def _build_nc():
    dt = mybir.dt
    nc = bacc.Bacc("TRN2", debug=False, target_bir_lowering=False)
    x_d = nc.dram_tensor("x", [M_SH, K], dt.float32, kind="ExternalInput")
    w_d = nc.dram_tensor("w", [N_SH, K], dt.float32, kind="ExternalInput")
    b_d = nc.dram_tensor("b", [128, N_SH], dt.float32, kind="ExternalInput")
    y_d = nc.dram_tensor("y", [M_SH, N_SH], dt.bfloat16, kind="ExternalOutput")
    warm_d = nc.dram_tensor("warm", [1, 128], dt.bfloat16, kind="ExternalOutput")

    with tile.TileContext(nc) as tc:
        with (
            tc.tile_pool(name="sb", bufs=1) as pool,
            tc.tile_pool(name="ps", bufs=2, space=bass.MemorySpace.PSUM) as psum,
            tc.tile_pool(name="acc", bufs=1, space=bass.MemorySpace.PSUM) as psacc,
        ):
            # identity built on-chip (gpsimd is otherwise idle, so this
            # completes ~3us before any DMA data): 0-fill, 1.0 diagonal
            idt = pool.tile([128, 128], dt.bfloat16, tag="idt")
            nc.gpsimd.memset(idt[:, :], 0.0)
            nc.gpsimd.affine_select(
                idt[:, :], idt[:, :], [[1, 128]],
                compare_op=mybir.AluOpType.not_equal, fill=1.0,
                base=0, channel_multiplier=-1,
            )

            # PE warmup: dummy transposes with no DMA deps — they run during
            # the load phase and hold the HAM clock gate open.  Kept alive
            # via a tiny DMA'd output.
            wps = psum.tile([128, 128], dt.bfloat16, tag="wps", bufs=1)
            for _ in range(N_WARM):
                nc.tensor.transpose(wps[:, :], idt[:, :], idt[:, :])
            wsb = pool.tile([1, 128], dt.bfloat16, tag="wsb")
            nc.vector.tensor_copy(wsb[0:1, :], wps[0:1, :])
            nc.scalar.dma_start(out=warm_d[:, :], in_=wsb[0:1, :])

            # loads: sync- and scalar-issued HWDGE DMAs use different queue
            # rings that run concurrently (~200 GB/s each) — split each
            # operand across both rings by row-tile, x before w
            xf = pool.tile([128, RT, K], dt.float32, tag="xf")
            wf = pool.tile([128, RT, K], dt.float32, tag="wf")
            x_src = x_d.ap().rearrange("(t p) k -> p t k", p=128)
            w_src = w_d.ap().rearrange("(t p) k -> p t k", p=128)
            nc.sync.dma_start(out=xf[:, 0:1, :], in_=x_src[:, 0:1, :])
            nc.scalar.dma_start(out=xf[:, 1:2, :], in_=x_src[:, 1:2, :])
            nc.sync.dma_start(out=wf[:, 0:1, :], in_=w_src[:, 0:1, :])
            nc.scalar.dma_start(out=wf[:, 1:2, :], in_=w_src[:, 1:2, :])
            bias_all = pool.tile([128, N_SH], dt.float32, tag="bias_all")
            nc.sync.dma_start(out=bias_all[:, :], in_=b_d[:, :])

            # hi-u16 lane views = truncated-bf16 bit patterns
            xhi = xf[:, :, :].bitcast(dt.bfloat16).rearrange(
                "p t (k two) -> p t k two", two=2
            )
            whi = wf[:, :, :].bitcast(dt.bfloat16).rearrange(
                "p t (k two) -> p t k two", two=2
            )

            # transpose hi-lanes to K-partition-major; phase-ordered so each
            # K-half's work starts as soon as its DMA lands
            xt = [None] * (KB // 2)
            wt = [None] * (KB // 2)
            acc = [
                psacc.tile([128, N_SH], dt.float32, tag=f"acc{mb}", name=f"acc{mb}")
                for mb in range(RT)
            ]

            # kb-pair batched transposes: 4 PE transposes per PSUM tile and
            # ONE DVE copy per pair (halves the DVE per-op overhead)
            def transpose_pair(kp, hi_view, dst_list, tag, masked):
                tk = pool.tile([128, 2, RT, 128], dt.bfloat16,
                               tag=f"{tag}{kp}", name=f"{tag}{kp}")
                pt = psum.tile([128, 2, RT, 128], dt.bfloat16, tag=f"pt_{tag}",
                               name=f"pt_{tag}{kp}", bufs=2)
                for i in range(2):
                    kb = kp * 2 + i
                    for t in range(RT):
                        nc.tensor.transpose(
                            pt[:, i, t, :],
                            hi_view[:, t, kb * 128:(kb + 1) * 128, 1],
                            idt[:, :],
                        )
                if masked:
                    # fused copy + BF15 mask (clear mantissa bit 7)
                    nc.vector.tensor_scalar(
                        out=tk[:, :, :, :].bitcast(dt.uint16),
                        in0=pt[:, :, :, :].bitcast(dt.uint16),
                        scalar1=0xFFFE, scalar2=None,
                        op0=mybir.AluOpType.bitwise_and,
                    )
                else:
                    nc.vector.tensor_copy(tk[:, :, :, :], pt[:, :, :, :])
                dst_list[kp] = tk

            for kp in range(KB // 2):
                transpose_pair(kp, xhi, xt, "xt", masked=True)
            # second dummy batch: if the W data is late (DMA contention), the
            # PE would idle long enough for the HAM clock gate to drop back
            # to 1.2 GHz right before the matmul tail — keep it busy
            for _ in range(10):
                nc.tensor.transpose(wps[:, :], idt[:, :], idt[:, :])
            for kp in range(KB // 2):
                transpose_pair(kp, whi, wt, "wt", masked=False)
            for kb in range(KB):
                kp, i = divmod(kb, 2)
                for mb in range(RT):
                    nc.tensor.matmul(
                        acc[mb][:, :],
                        xt[kp][:, i, mb, :],
                        wt[kp][:, i, :, :],
                        start=(kb == 0),
                        stop=(kb == KB - 1),
                    )

            # epilogue + store, one per M-block on separate trigger queues
            ysb = pool.tile([128, RT, N_SH], dt.bfloat16, tag="ysb")
            y_dst = y_d.ap().rearrange("(mb p) n -> p mb n", p=128)
            for mb in range(RT):
                nc.vector.tensor_tensor(
                    out=ysb[:, mb, :], in0=acc[mb][:, :], in1=bias_all[:, :],
                    op=mybir.AluOpType.add,
                )
                eng = nc.scalar if mb == 0 else nc.sync
                eng.dma_start(out=y_dst[:, mb, :], in_=ysb[:, mb, :])

    nc.compile()
    return nc


def get_nc():
    if "nc" not in _CACHE:
        _CACHE["nc"] = _build_nc()
    return _CACHE["nc"]


def make_in_maps(x: np.ndarray, weight: np.ndarray, bias: np.ndarray):
    x2d = np.ascontiguousarray(np.asarray(x).reshape(M, K), dtype=np.float32)
    w = np.ascontiguousarray(np.asarray(weight), dtype=np.float32)
    b = np.ascontiguousarray(np.asarray(bias), dtype=np.float32)
    in_maps = []
    for c in range(M_GROUPS * N_GROUPS):
        mi, ni = divmod(c, N_GROUPS)
        bs = np.ascontiguousarray(
            np.broadcast_to(b[ni * N_SH:(ni + 1) * N_SH], (128, N_SH))
        )
        in_maps.append({
            "x": np.ascontiguousarray(x2d[mi * M_SH:(mi + 1) * M_SH]),
            "w": np.ascontiguousarray(w[ni * N_SH:(ni + 1) * N_SH]),
            "b": bs,
        })
    return in_maps


def assemble(results) -> np.ndarray:
    y2d = np.empty((M, N), dtype=ml_dtypes.bfloat16)
    for c in range(M_GROUPS * N_GROUPS):
        mi, ni = divmod(c, N_GROUPS)
        y2d[mi * M_SH:(mi + 1) * M_SH, ni * N_SH:(ni + 1) * N_SH] = results[c]["y"]
    return y2d.reshape(4, 128, N)


def kernel(x: np.ndarray, weight: np.ndarray, bias: np.ndarray) -> np.ndarray:
    nc = get_nc()
    in_maps = make_in_maps(x, weight, bias)
    res = run_bass_kernel_spmd(nc, in_maps, core_ids=list(range(8)))
    return assemble(res.results)
